# revision 42
# baseline (speedup 1.0000x reference)
"""Causal self-attention (B=2, T=2048, C=768, H=12, DH=64) on 8 TRN2 cores.

Sharding: core = (b, g), b in {0,1} batch, g in {0..3} head-group of 3
heads.  Data parallel on B, tensor parallel on H: Wqkv column-sharded,
Wout row-sharded; the 4 partial outputs per batch are summed on the host
(the all-reduce of the row-parallel projection).

Device kernel (per core), bf16 compute / f32 PSUM:
  - qkT [384, T] = Wqk_shard.T @ x.T with W col order [q0 q2|k0 k2|q1 k1]
    so heads 0/2 occupy the two 64-row PE lanes (concurrent score MMs);
    head 1 gets a partition-swapped copy (qk_s[3]) for lane alternation.
  - V' is computed directly in [t, dh] orientation; col 64 of each
    65-wide block is 1.0 (ones column -> softmax denominator).
  - scores are computed transposed S^T [k, q], causal-chunked; exp on
    ScalarE reads PSUM and writes P^T bf16 tiles.
  - AV is computed TRANSPOSED: stationary V' [128, 65] (65-col weight
    load, reused across a whole q-block stream), moving P^T [128, <=512]
    -> poT [65, q] accumulates in PSUM over k-tiles.  Row 64 = softmax
    denominator d[q].  This removes the 128-col LDWEIGHTS per AV matmul
    (the old LDW-bound AV) and all PE transposes in the projection.
  - normalization per (head, 4-j block): d row -> SBUF -> DMA-scatter to
    [16, 32] -> DVE reciprocal (lane-parallel) -> DMA-gather back to a
    row -> GPSIMD partition_broadcast [64, 512] -> one DVE multiply
    gives aT [64, q] bf16, the projection stationary.
  - projection per q-tile: 3 per-head K=64 matmuls (aT_h stationary,
    Wout_h rows moving) accumulate into [q, 512] + [q, 256] PSUM;
    DVE copies to SBUF; DMA out.
"""

import os
import sys

sys.path.insert(0, "/root/.axon_site")
sys.path.insert(0, "/root/.axon_site/_ro/trn_rl_repo")
sys.path.insert(0, "/root/.axon_site/_ro/pypackages")

import numpy as np
import ml_dtypes

import concourse.bass as bass
import concourse.mybir as mybir
import concourse.tile as tile
import concourse.bacc as bacc
from concourse.bass_utils import run_bass_kernel_spmd

B, T, C, H, DH = 2, 2048, 768, 12, 64
G = 4                 # head groups (tensor parallel)
HPG = H // G          # 3 heads per group
CPG = HPG * DH        # 192 output cols per group
NCORES = B * G        # 8

F32 = mybir.dt.float32
BF16 = mybir.dt.bfloat16

NT = T // 128         # 16 t-tiles
NCS = C // 128        # 6 c-strips
# q-blocks (q0, width): last 512 split in two so the final AV^T units'
# normalization chains overlap the exp tail instead of serializing
BLKS = [(0, 512), (512, 512), (1024, 512), (1536, 256), (1792, 128),
        (1920, 128)]
SCALE = DH ** -0.5

_COMPILED = {}


def _patch_walrus_ldw_opt():
    """Enable walrus's LDWEIGHTS elision: consecutive matmuls with the
    same stationary operand skip the redundant weight reloads."""
    from concourse import bass_utils
    if getattr(bass_utils, "_ldw_opt_patched", False):
        return
    orig = bass_utils.run_command

    def patched(cmd, **kw):
        cmd = ["--enable-ldw-opt=true" if c == "--enable-ldw-opt=false" else c
               for c in cmd]
        return orig(cmd, **kw)

    bass_utils.run_command = patched
    bass_utils._ldw_opt_patched = True


def _build_nc():
    from contextlib import ExitStack

    if os.environ.get("LDW_OPT", "0") == "1":
        _patch_walrus_ldw_opt()
    nc = bacc.Bacc("TRN2", debug=False, num_devices=NCORES)

    xt_d = nc.dram_tensor("xt", [C, T], BF16, kind="ExternalInput").ap()
    wqk_d = nc.dram_tensor("wqk", [C, 384], BF16, kind="ExternalInput").ap()
    wv_d = nc.dram_tensor("wv", [C, CPG], BF16, kind="ExternalInput").ap()
    wo_d = nc.dram_tensor("wo", [128, 2 * C], BF16, kind="ExternalInput").ap()
    mk_d = nc.dram_tensor("mask", [128, 128], BF16, kind="ExternalInput").ap()
    out_d = nc.dram_tensor("out", [T, C], F32, kind="ExternalOutput").ap()

    with tile.TileContext(nc) as tc, ExitStack() as ctx:
        _kernel(ctx, tc, nc, xt_d, wqk_d, wv_d, wo_d, mk_d, out_d)

    nc.compile()
    return nc


def _kernel(ctx, tc, nc, xt_d, wqk_d, wv_d, wo_d, mk_d, out_d):
    Exp = mybir.ActivationFunctionType.Exp

    # ---- persistent SBUF tensors -------------------------------------
    persist = ctx.enter_context(tc.tile_pool(name="persist", bufs=1))

    def single(shape, dtype, name):
        return persist.tile(shape, dtype, tag=name, name=name)

    xt_s = [single([128, T], BF16, f"xt{i}") for i in range(NCS)]
    wqk_s = [single([128, 384], BF16, f"wqk{i}") for i in range(NCS)]
    wv_s = [single([128, CPG], BF16, f"wv{i}") for i in range(NCS)]
    # [128, 1536]: [Wout_h0; Wout_h1] | [Wout_h2; zeros]
    wo_s = single([128, 2 * C], BF16, "wo")
    maskt = single([128, 128], BF16, "maskt")
    # qkT m-tiles: qk_s[0]=[q0|q2] qk_s[1]=[k0|k2] qk_s[2]=[q1|k1]
    # qk_s[3]=[k1|q1] (partition-swapped copy of qk_s[2])
    qk_s = [single([128, T], BF16, f"qk{i}") for i in range(4)]
    # V' per head: 65 cols per k-tile, col 64 = 1.0 (softmax denom)
    VOFF = [0, 1040, 2080]
    vp_all = single([128, 3 * 1040], BF16, "vpall")

    # ---- input DMAs: interleave strips so cs=0 inputs land first -----
    qdma = [nc.sync, nc.gpsimd]

    for cs in range(NCS):
        qdma[cs % 2].dma_start(wqk_s[cs][:, :], wqk_d[cs * 128:(cs + 1) * 128, :])
        qdma[(cs + 1) % 2].dma_start(
            xt_s[cs][:, 0:512], xt_d[cs * 128:(cs + 1) * 128, 0:512])
    for n4 in range(1, 4):
        for cs in range(NCS):
            qdma[(cs + n4) % 2].dma_start(
                xt_s[cs][:, n4 * 512:(n4 + 1) * 512],
                xt_d[cs * 128:(cs + 1) * 128, n4 * 512:(n4 + 1) * 512])
    nc.sync.dma_start(maskt[:, :], mk_d[:, :])
    for cs in range(NCS):
        qdma[cs % 2].dma_start(wv_s[cs][:, :], wv_d[cs * 128:(cs + 1) * 128, :])
    nc.sync.dma_start(wo_s[:, :], wo_d[:, :])
    nc.gpsimd.memset(vp_all[:, :], 1.0)
    warm = single([128, 512], BF16, "warm")
    nc.vector.memset(warm[:, :], 0.0)

    # ---- PSUM pools (8 banks of 2KB/partition total) -----------------
    # scp: score chunks, [128,1024] f32 ring of 2           -> 4 banks
    # avt: AV^T accumulators [65,512] (padded), ring of 2   -> 2 banks
    # big: qkT chunks / direct-V / proj halves, ring of 2   -> 2 banks
    scp = ctx.enter_context(tc.tile_pool(name="scp", bufs=2, space="PSUM"))
    avt = ctx.enter_context(tc.tile_pool(name="avt", bufs=2, space="PSUM"))
    big = ctx.enter_context(tc.tile_pool(name="pbig", bufs=2, space="PSUM"))
    atp = ctx.enter_context(tc.tile_pool(name="atp", bufs=6))
    otp = ctx.enter_context(tc.tile_pool(name="otp", bufs=3))
    nrm = ctx.enter_context(tc.tile_pool(name="nrm", bufs=3))

    nn = [0]

    def psum_sc(p, f):
        t = scp.tile([p, f], F32, tag="sc", name=f"sc{nn[0]}",
                     padded_shape=[128, 1024])
        nn[0] += 1
        return t

    def psum_avt():
        t = avt.tile([128, 512], F32, tag="avt", name=f"av{nn[0]}",
                     padded_shape=[128, 512])
        nn[0] += 1
        return t

    def psum_big(p, f):
        t = big.tile([p, f], F32, tag="big", name=f"bg{nn[0]}",
                     padded_shape=[128, 512])
        nn[0] += 1
        return t

    # ---- qkT: m-tile chunk = 6-strip accumulation + PSUM->SBUF copy --
    def emit_qk_chunk(m, n4):
        ps = psum_big(128, 512)
        for cs in range(NCS):
            nc.tensor.matmul(
                ps[:, :],
                wqk_s[cs][:, m * 128:(m + 1) * 128],
                xt_s[cs][:, n4 * 512:(n4 + 1) * 512],
                start=(cs == 0), stop=(cs == NCS - 1),
            )
        nc.vector.tensor_copy(qk_s[m][:, n4 * 512:(n4 + 1) * 512], ps[:, :])

    # ---- direct V: V[t, dh] = xT-chunk.T @ Wv ------------------------
    def emit_v(t):
        pv = psum_big(128, CPG)
        for cs in range(NCS):
            nc.tensor.matmul(
                pv[:, :],
                xt_s[cs][:, t * 128:(t + 1) * 128],
                wv_s[cs][:, :],
                start=(cs == 0), stop=(cs == NCS - 1),
            )
        # one strided copy: head h block of 64 cols -> vp col 1040h+65t
        dst = vp_all[:, :].rearrange("p (h f) -> p h f", h=HPG)[:, :, 65 * t:65 * t + 64]
        src_ = pv[:, :].rearrange("p (h c) -> p h c", h=HPG)
        nc.vector.tensor_copy(dst, src_)

    # ---- scores lanes ------------------------------------------------
    # lane 0 = array rows 0:64, lane 1 = rows 64:128 (via base partition)
    q_loc = {0: (0, 0), 2: (0, 64), "1lo": (2, 0), "1hi": (3, 64)}
    k_loc = {0: (1, 0), 2: (1, 64), "1lo": (3, 0), "1hi": (2, 64)}

    pt_all = [[None] * NT for _ in range(HPG)]

    def emit_scores_chunk(h, i, ci):
        qlen = T - 128 * i
        pti = pt_all[h][i]
        q0 = 128 * i
        c0 = ci * 1024
        L = min(1024, qlen - c0)
        if h == 1:
            key = "1lo" if ci % 2 == 0 else "1hi"
        else:
            key = h
        qt, qp = q_loc[key]
        kt, kp = k_loc[key]
        sc = psum_sc(128, L)
        for s0 in range(0, L, 512):
            sl = min(512, L - s0)
            nc.tensor.matmul(
                sc[:, s0:s0 + sl],
                qk_s[kt][kp:kp + 64, i * 128:(i + 1) * 128],
                qk_s[qt][qp:qp + 64, q0 + c0 + s0:q0 + c0 + s0 + sl],
                start=True, stop=True,
            )
        nc.scalar.activation(pti[:, c0:c0 + L], sc[:, :], Exp, scale=SCALE)
        if ci == 0:
            # zero the upper-triangular (k > q) part of the diag block
            nc.vector.tensor_mul(pti[:, 0:128], pti[:, 0:128], maskt[:, :])

    def emit_scores(h, i, chunks=None):
        qlen = T - 128 * i
        if pt_all[h][i] is None:
            pt_all[h][i] = single([128, qlen], BF16, f"pth{h}i{i}")
        nch = (qlen + 1023) // 1024
        for ci in (range(nch) if chunks is None else chunks):
            if ci < nch:
                emit_scores_chunk(h, i, ci)

    # ---- AV^T unit (h, blk): poT [65, w] over q in [q0, q0+w) -------
    # aPair[blk] [128, w]: rows 0:64 = aT_h0, rows 64:128 = aT_h1;
    # a2[blk] [64, w] = aT_h2 (proj pads its contraction with zero rows)
    aPair = [None] * len(BLKS)
    a2 = [None] * len(BLKS)

    def make_avt(h, blk):
        """Return (mm_items, chain_item) for unit (h, blk): one emission
        closure per matmul so they can interleave with score chunks."""
        state = {}
        q0, w = BLKS[blk]
        i0 = q0 // 128
        last_i = (q0 + w) // 128 - 1

        def mk(i):
            def f():
                if "po" not in state:
                    state["po"] = psum_avt()
                po = state["po"]
                pc0 = 0 if i < i0 else 128 * i - q0
                pts = pt_all[h][i][:, q0 + pc0 - 128 * i: q0 + w - 128 * i]
                nc.tensor.matmul(
                    po[0:65, pc0:w],
                    vp_all[:, VOFF[h] + 65 * i: VOFF[h] + 65 * i + 65],
                    pts, start=(i == 0), stop=(i == last_i),
                )
            return f

        def chain():
            # normalization: full PSUM->SBUF copy first (frees the avt
            # bank after one DVE op; the slow d-chain runs from SBUF)
            po = state["po"]
            nsc = w // 16  # scatter cols per partition (16 partitions)
            poc = nrm.tile([65, 512], F32, tag="poc", name=f"pc{h}_{blk}")
            nc.vector.tensor_copy(poc[0:65, 0:w], po[0:65, 0:w])
            dal = nrm.tile([16, 32], F32, tag="dal", name=f"da{h}_{blk}")
            nc.sync.dma_start(dal[0:16, 0:nsc], poc[64:65, 0:w])
            ral = nrm.tile([16, 32], F32, tag="ral", name=f"ra{h}_{blk}")
            nc.vector.reciprocal(ral[0:16, 0:nsc], dal[0:16, 0:nsc])
            rrow = nrm.tile([1, 512], F32, tag="rrow", name=f"rr{h}_{blk}")
            nc.sync.dma_start(rrow[0:1, 0:w], ral[0:16, 0:nsc])
            bc = nrm.tile([64, 512], F32, tag="bc", name=f"bc{h}_{blk}")
            nc.gpsimd.partition_broadcast(bc[0:64, 0:w], rrow[0:1, 0:w],
                                          channels=64)
            state["poc"], state["bc"] = poc, bc
        def mul_item():
            # deferred one unit in DVE program order: by now the bcast
            # round-trip has completed, so this never head-of-line
            # blocks the next unit's poc copy in the DVE FIFO
            poc, bc = state["poc"], state["bc"]
            if aPair[blk] is None:
                aPair[blk] = atp.tile([128, 512], BF16, tag="aP",
                                      name=f"aP{blk}")
            if h == 0:
                nc.vector.tensor_mul(aPair[blk][0:64, 0:w],
                                     poc[0:64, 0:w], bc[0:64, 0:w])
            elif h == 1:
                # mul in-lane at 0:64, then cross-partition SBUF DMA
                # into the pair tile's upper half
                a1t = nrm.tile([64, 512], BF16, tag="a1t", name=f"a1t{blk}")
                nc.vector.tensor_mul(a1t[0:64, 0:w], poc[0:64, 0:w],
                                     bc[0:64, 0:w])
                nc.sync.dma_start(aPair[blk][64:128, 0:w], a1t[0:64, 0:w])
            else:
                a2[blk] = atp.tile([128, 512], BF16, tag="a2",
                                   name=f"a2_{blk}")
                # rows 64:128 must be finite: they meet zero Wout rows
                nc.gpsimd.memset(a2[blk][64:128, 0:w], 0.0)
                nc.vector.tensor_mul(a2[blk][0:64, 0:w], poc[0:64, 0:w],
                                     bc[0:64, 0:w])

        return [mk(i) for i in range(last_i + 1)], chain, mul_item

    # ---- projection per q-tile j (needs aT of its block) -------------
    def blk_of(j):
        for k, (q0, w) in enumerate(BLKS):
            if q0 <= 128 * j < q0 + w:
                return k, (128 * j - q0) // 128
        raise AssertionError

    def emit_proj(j):
        blk, r = blk_of(j)
        pa = psum_big(128, 512)
        pb = psum_big(128, 256)
        sl = slice(128 * r, 128 * r + 128)
        # wo_s [128, 1536]: cols 0:768 rows 0:64 = Wout_h0, rows 64:128
        # = Wout_h1; cols 768:1536 rows 0:64 = Wout_h2, rows 64:128 = 0.
        # h2's stationary is zero-padded to K=128 (uniform tile sizes --
        # mixed-row-tile accumulation groups fault on HW).
        sp = aPair[blk][:, sl]
        s2 = a2[blk][:, sl]
        nc.tensor.matmul(pa[:, :], sp, wo_s[:, 0:512],
                         start=True, stop=False)
        nc.tensor.matmul(pa[:, :], s2, wo_s[:, C:C + 512],
                         start=False, stop=True)
        nc.tensor.matmul(pb[:, :], sp, wo_s[:, 512:768],
                         start=True, stop=False)
        nc.tensor.matmul(pb[:, :], s2, wo_s[:, C + 512:2 * C],
                         start=False, stop=True)
        ot = otp.tile([128, C], F32, tag="ot", name=f"ot{j}")
        if j >= 8:
            nc.scalar.copy(ot[:, 0:512], pa[:, :])
            nc.scalar.copy(ot[:, 512:768], pb[:, :])
        else:
            nc.vector.tensor_copy(ot[:, 0:512], pa[:, :])
            nc.vector.tensor_copy(ot[:, 512:768], pb[:, :])
        nc.gpsimd.dma_start(out_d[j * 128:(j + 1) * 128, :], ot[:, :])

    # ---- emission order = scheduler priority -------------------------
    def nch(i):
        return (T - 128 * i + 1023) // 1024

    def interleave(A, B):
        out = []
        na, nb = len(A), len(B)
        ia = ib = 0
        while ia < na or ib < nb:
            if ib >= nb or (ia < na and ia * max(nb, 1) <= ib * max(na, 1)):
                out.append(A[ia]); ia += 1
            else:
                out.append(B[ib]); ib += 1
        return out

    # HAM warm-up: keep the PE busy while input DMAs land so the clock
    # gate opens (K=8/8) before the first real matmuls
    for wi in range(14):
        wp = avt.tile([128, 512], F32, tag="avt", name=f"warm{wi}",
                      padded_shape=[128, 512])
        nc.tensor.matmul(wp[:, :], warm[:, 0:128], warm[:, :],
                         start=True, stop=True)
    emit_qk_chunk(0, 0)
    emit_qk_chunk(0, 1)
    emit_qk_chunk(1, 0)
    emit_scores(0, 0, chunks=[0])
    emit_scores(2, 0, chunks=[0])
    emit_qk_chunk(0, 2)
    emit_qk_chunk(0, 3)
    emit_scores(0, 0, chunks=[1])
    emit_scores(2, 0, chunks=[1])
    emit_scores(0, 1)
    emit_scores(2, 1)
    for n4 in range(1, 4):
        emit_qk_chunk(1, n4)
    emit_scores(0, 2)
    emit_scores(2, 2)
    for n4 in range(4):
        emit_qk_chunk(2, n4)
    # qk_s[3] = [k1|q1]: partition-swapped copy (SBUF->SBUF DMA)
    nc.gpsimd.dma_start(qk_s[3][0:64, :], qk_s[2][64:128, :])
    nc.gpsimd.dma_start(qk_s[3][64:128, :], qk_s[2][0:64, :])
    emit_v(0)
    emit_v(1)
    emit_scores(0, 3)
    emit_scores(2, 3)
    emit_v(2)
    emit_v(3)
    for i in range(4):
        emit_scores(1, i)

    # block loop: exp-pacing score chunks for block blk+1 interleaved
    # one-by-one with AV^T matmuls of block blk, V of block blk+1, and
    # the projections of block blk-1 (aT chain latency to hide).
    pending_proj = []
    pending_mul = []
    for blk in range(len(BLKS)):
        q0, w = BLKS[blk]
        last_i = (q0 + w) // 128 - 1
        nxt_last = ((BLKS[blk + 1][0] + BLKS[blk + 1][1]) // 128 - 1
                    if blk + 1 < len(BLKS) else last_i)
        fillers = []
        for i in range(last_i + 1, nxt_last + 1):
            for ci in range(nch(i)):
                def f02(i=i, ci=ci):
                    emit_scores(0, i, chunks=[ci])
                    emit_scores(2, i, chunks=[ci])
                fillers.append(f02)
        for i in range(last_i + 1, nxt_last + 1):
            for ci in range(nch(i)):
                fillers.append(
                    lambda i=i, ci=ci: emit_scores(1, i, chunks=[ci]))
        mms = []
        vts = [lambda t=t: emit_v(t) for t in range(last_i + 1, nxt_last + 1)]
        for h in range(HPG):
            if pending_mul:
                mms.append(pending_mul.pop(0))
            if pending_proj:
                mms.append(pending_proj.pop(0))
            if vts:
                mms.append(vts.pop(0))
            mm_items, chain_item, mul_item = make_avt(h, blk)
            mms += mm_items
            mms.append(chain_item)
            pending_mul.append(mul_item)
        mms = vts + mms + pending_proj
        pending_proj = [lambda j=j: emit_proj(j)
                        for j in range(q0 // 128, (q0 + w) // 128)]
        for it in interleave(fillers, mms):
            it()
    for it in pending_mul + pending_proj:
        it()


def get_nc():
    if "nc" not in _COMPILED:
        _COMPILED["nc"] = _build_nc()
    return _COMPILED["nc"]


def make_in_maps(x, Wqkv, Wout):
    """Host-side sharding: one input map per core (core = b*G + g)."""
    x = np.asarray(x, dtype=np.float32)
    Wqkv = np.asarray(Wqkv, dtype=np.float32)
    Wout = np.asarray(Wout, dtype=np.float32)

    # mask[k, q] = 1 where k <= q  (valid causal entries of the diag block)
    mask = np.triu(np.ones((128, 128), dtype=np.float32)).astype(
        ml_dtypes.bfloat16)

    in_maps = []
    for b in range(B):
        xt = np.ascontiguousarray(x[b].T).astype(ml_dtypes.bfloat16)
        for g in range(G):
            h0, h1, h2 = (g * HPG + hh for hh in range(HPG))

            def col(kind, hd):
                base = {"q": 0, "k": C, "v": 2 * C}[kind]
                return Wqkv[:, base + hd * DH: base + (hd + 1) * DH]

            # m-tiles: [q0 q2 | k0 k2 | q1 k1]
            wqk = np.concatenate([
                col("q", h0), col("q", h2),
                col("k", h0), col("k", h2),
                col("q", h1), col("k", h1),
            ], axis=1).astype(ml_dtypes.bfloat16)
            wv = np.concatenate(
                [col("v", hd) for hd in (h0, h1, h2)], axis=1,
            ).astype(ml_dtypes.bfloat16)
            # wo [128, 1536]: [Wout_h0; Wout_h1] | [Wout_h2; zeros]
            wo = np.zeros((128, 2 * C), dtype=np.float32)
            wo[0:64, 0:C] = Wout[h0 * DH:(h0 + 1) * DH, :]
            wo[64:128, 0:C] = Wout[h1 * DH:(h1 + 1) * DH, :]
            wo[0:64, C:2 * C] = Wout[h2 * DH:(h2 + 1) * DH, :]
            wo = wo.astype(ml_dtypes.bfloat16)
            in_maps.append({
                "xt": xt, "wqk": np.ascontiguousarray(wqk),
                "wv": np.ascontiguousarray(wv),
                "wo": np.ascontiguousarray(wo),
                "mask": mask,
            })
    return in_maps


def kernel(x, Wqkv, Wout):
    nc = get_nc()
    in_maps = make_in_maps(x, Wqkv, Wout)
    res = run_bass_kernel_spmd(nc, in_maps, list(range(NCORES))).results
    out = np.zeros((B, T, C), dtype=np.float32)
    for b in range(B):
        for g in range(G):
            out[b] += res[b * G + g]["out"]
    return out


if __name__ == "__main__":
    nc = get_nc()
    print("built + compiled ok")
